# revision 34
# baseline (speedup 1.0000x reference)
"""Multi-head causal self-attention (B=2, S=2048, E=1024, H=16, D=64) on 8 TRN2
NeuronCores.

Sharding: core c owns batch b = c//4 and head-group g = c%4 (4 heads each).
Per core, transpose-free layout:
  QT/KT [d_local=256, S] (d on partitions), V [S, d_local] (t on partitions),
  scoresT [t, s] blocks via lhsT=KT-block; softmax is unnormalized exp (f32).

Optimizations vs the original 191us baseline (now ~158us):
  - AV uses column-packed matmul pairs: heads (h, h+1) of a pair run as two
    M=64 matmuls at tile_position (0,0)/(0,64) into one PSUM bank (rows
    0:64 / 64:128), concurrent in the PE array.  Row sums come from four
    M=1 matmuls (ones lhsT) col-packed at positions 0/32/64/96 into a single
    sums bank.  Accumulator banks are zero-memset and all AV/sum matmuls use
    start=False (accumulate-onto-zero), robust to has_written granularity.
  - Normalization per pair: one bf16 copy of the sums bank, two row-packed
    bf16 K=1 broadcast matmuls reusing the sums bank, one reciprocal and one
    multiply covering both heads [128,512].  This replaces the baseline's
    f32 1*64*512 broadcast matmuls (21.5us of PE) and the ScalarE copies.
  - PSUM pooled in three phases (QKV / attention / tail-proj) to fit the
    8-bank budget: scores 2x[128,1024]x2bufs (4) + AV pairs (2) + sums (1)
    + one shared transient bank for proj-drip / V units.
  - V t-blocks 4..15 are built at attention block boundaries where the PE
    would otherwise idle during DVE normalization; proj for block i drips
    one unit per j-iteration into block i+1's attention loop.
  - Output partials are written in bf16 (halves out-DMA; host sums in f32).

Weights/activations are cast to bf16 host-side (matmul inputs); all
accumulation is f32 in PSUM; softmax exp/normalization in f32.
"""

import numpy as np
import ml_dtypes

import concourse.bass as bass
import concourse.tile as tile
from concourse import bacc, mybir
from concourse import bass_utils

B, S, E, H, D = 2, 2048, 1024, 16, 64
NCORES = 8
HPC = 4                 # heads per core
EL = HPC * D            # 256 local channels
SBW = 512               # s-block width
NSB = S // SBW          # 4
TBW = 128               # t-block width
NTB = S // TBW          # 16
NEB = E // 128          # 8 e-blocks
SCALE = 1.0 / np.sqrt(D)

F32 = mybir.dt.float32
BF16 = mybir.dt.bfloat16

_BUILT = None


def _emit(tc, nc, d):
    Exp = mybir.ActivationFunctionType.Exp
    Ident = mybir.ActivationFunctionType.Identity

    with (
        tc.tile_pool(name="const", bufs=1) as cst,
        tc.tile_pool(name="big", bufs=1) as big,
        tc.tile_pool(name="ptp", bufs=8) as ptp,
        tc.tile_pool(name="rsp", bufs=2) as rsp,
        tc.tile_pool(name="bcsp", bufs=2) as bcsp,
        tc.tile_pool(name="outp", bufs=4) as outp,
    ):
        # ---- load inputs (order = arrival priority) ----
        wq = big.tile([128, NEB * EL], BF16, name="wq", tag="wq")
        wq3 = wq.rearrange("p (j t c) -> p j t c", t=2, c=128)
        wqd3 = d["wq"].rearrange("p (j t c) -> p j t c", t=2, c=128)
        nc.sync.dma_start(wq3[:, :, 0:1], wqd3[:, :, 0:1])
        bq = cst.tile([128, 2], F32, name="bq", tag="bq")
        nc.sync.dma_start(bq[:], d["bq"][:])
        # x^T e-block tiles, DMA'd in s-block chunks so compute starts early
        xt = [big.tile([128, S], BF16, name=f"xt{j}", tag=f"xt{j}")
              for j in range(NEB)]
        for j in range(NEB):
            nc.sync.dma_start(
                xt[j][:, 0:SBW], d["xt"][:, j * S: j * S + SBW]
            )
        nc.sync.dma_start(wq3[:, :, 1:2], wqd3[:, :, 1:2])
        wk = big.tile([128, NEB * EL], BF16, name="wk", tag="wk")
        nc.sync.dma_start(wk[:], d["wk"][:])
        bk = cst.tile([128, 2], F32, name="bk", tag="bk")
        nc.sync.dma_start(bk[:], d["bk"][:])
        for i in range(1, NSB):
            for j in range(NEB):
                nc.sync.dma_start(
                    xt[j][:, i * SBW:(i + 1) * SBW],
                    d["xt"][:, j * S + i * SBW: j * S + (i + 1) * SBW],
                )
        wv = big.tile([128, NEB * EL], BF16, name="wv", tag="wv")
        nc.sync.dma_start(wv[:], d["wv"][:])
        bv = cst.tile([128, EL], F32, name="bv", tag="bv")
        nc.sync.dma_start(bv[:], d["bv"][:])
        wp = big.tile([128, 2 * E], BF16, name="wp", tag="wp")
        nc.sync.dma_start(wp[:], d["wp"][:])
        tri = cst.tile([128, 128], BF16, name="tri", tag="tri")
        nc.sync.dma_start(tri[:], d["tri"][:])
        ones_bc = cst.tile([128, 64], BF16, name="ones_bc", tag="onbc")
        nc.vector.memset(ones_bc[:], 1.0)
        ones_sum = cst.tile([128, 1], BF16, name="ones_sum", tag="onsm")
        nc.vector.memset(ones_sum[:], 1.0)

        # V tiles [128, 256]: head h at cols 64h..64h+64
        vt = [big.tile([128, EL], BF16, name=f"vt{j}", tag=f"vt{j}")
              for j in range(NTB)]

        qt = [big.tile([128, S], BF16, name=f"qt{k}", tag=f"qt{k}")
              for k in range(2)]
        kt = [big.tile([128, S], BF16, name=f"kt{k}", tag=f"kt{k}")
              for k in range(2)]
        yt = [big.tile([128, S], BF16, name=f"yt{k}", tag=f"yt{k}")
              for k in range(2)]

        # ---- phase A: QT/KT all blocks + V t-blocks 0..3 (rest dripped) ----
        with tc.tile_pool(name="qkvp", bufs=4, space="PSUM") as qkvp:
            for i in range(NSB):
                for dst, wl, bl in ((qt, wq, bq), (kt, wk, bk)):
                    for dt_i in range(2):
                        ac = qkvp.tile([128, SBW], F32, name="qk_ac", tag="qac")
                        for j in range(NEB):
                            nc.tensor.matmul(
                                ac[:],
                                wl[:, j * EL + dt_i * 128:
                                   j * EL + dt_i * 128 + 128],
                                xt[j][:, i * SBW:(i + 1) * SBW],
                                start=(j == 0),
                                stop=(j == NEB - 1),
                            )
                        nc.scalar.activation(
                            dst[dt_i][:, i * SBW:(i + 1) * SBW], ac[:], Ident,
                            bias=bl[:, dt_i:dt_i + 1], scale=1.0,
                        )
            for j16 in range(4):
                ac = qkvp.tile([128, SBW], F32, name="v_ac", tag="qac")
                for eb in range(NEB):
                    nc.tensor.matmul(
                        ac[:, 0:EL],
                        xt[eb][:, j16 * TBW:(j16 + 1) * TBW],
                        wv[:, eb * EL:(eb + 1) * EL],
                        start=(eb == 0),
                        stop=(eb == NEB - 1),
                    )
                nc.vector.tensor_add(vt[j16][:], ac[:, 0:EL], bv[:])

        # ---- phase B: attention per s-block, proj dripped one block behind --
        Copy = mybir.ActivationFunctionType.Copy
        n_ot = [0]

        def emit_proj_units(units, pool, tag, split_ot=False):
            for r0, nb2 in units:
                pr = pool.tile([128, 512], F32, name="pr", tag=tag,
                               space="PSUM")
                for cb in range(2):
                    nc.tensor.matmul(
                        pr[:],
                        yt[cb][:, r0:r0 + 128],
                        wp[:, cb * E + nb2 * 512: cb * E + (nb2 + 1) * 512],
                        start=(cb == 0),
                        stop=(cb == 1),
                    )
                ot = outp.tile([128, 512], BF16, name="ot", tag="ot")
                n_ot[0] += 1
                if split_ot and n_ot[0] % 2 == 0:
                    nc.scalar.activation(ot[:], pr[:], Copy)
                else:
                    nc.vector.tensor_copy(ot[:], pr[:])
                nc.sync.dma_start(
                    d["out"][r0:r0 + 128, nb2 * 512:(nb2 + 1) * 512], ot[:]
                )

        def emit_v_unit(j16):
            ac = trp.tile([128, 512], F32, name="v_ac", tag="tr",
                          space="PSUM")
            for eb in range(NEB):
                nc.tensor.matmul(
                    ac[:, 0:EL],
                    xt[eb][:, j16 * TBW:(j16 + 1) * TBW],
                    wv[:, eb * EL:(eb + 1) * EL],
                    start=(eb == 0),
                    stop=(eb == NEB - 1),
                )
            nc.vector.tensor_add(vt[j16][:], ac[:, 0:EL], bv[:])

        def emit_qk_unit(i1, dst, wl, bl, dt_i):
            # QT/KT for s-block i1, one d-tile; drains via DVE so the
            # attention phase keeps ScalarE exp-only
            ac = trp.tile([128, 512], F32, name="qk_ac", tag="tr",
                          space="PSUM")
            for jj in range(NEB):
                nc.tensor.matmul(
                    ac[:],
                    wl[:, jj * EL + dt_i * 128: jj * EL + dt_i * 128 + 128],
                    xt[jj][:, i1 * SBW:(i1 + 1) * SBW],
                    start=(jj == 0),
                    stop=(jj == NEB - 1),
                )
            nc.vector.tensor_scalar_add(
                dst[dt_i][:, i1 * SBW:(i1 + 1) * SBW], ac[:],
                bl[:, dt_i:dt_i + 1],
            )

        def emit_drip(unit):
            if unit[0] == "proj":
                emit_proj_units([unit[1]], trp, "tr")
            elif unit[0] == "v":
                emit_v_unit(unit[1])
            else:
                emit_qk_unit(*unit[1])

        proj_pending = []
        with (
            tc.tile_pool(name="accp", bufs=2, space="PSUM") as accp,
            tc.tile_pool(name="avsp", bufs=1, space="PSUM") as avsp,
            tc.tile_pool(name="trp", bufs=1, space="PSUM") as trp,
        ):
            for i in range(NSB):
                # drip queue: proj units of block i-1, plus (when the loop
                # is long enough) V units for block i+1
                drip = [("proj", u) for u in proj_pending]
                proj_pending = []
                vq = list(range(4 * (i + 1), min(4 * (i + 2), NTB)))
                if i >= 2 and 4 * i + 4 >= len(drip) + 4:
                    drip += [("v", j16) for j16 in vq]
                    vq = []
                avs = [avsp.tile([128, SBW], F32, name=f"avs{p}",
                                 tag=f"avs{p}") for p in range(2)]
                sums = avsp.tile([128, SBW], F32, name="sums", tag="sums")
                nc.vector.memset(avs[0][:], 0.0)
                nc.vector.memset(avs[1][:], 0.0)
                nc.vector.memset(sums[:], 0.0)
                njs = 4 * i + 4

                def av_mms(pts_, w_, j_):
                    lst = (j_ == njs - 1)

                    def av_pair(p):
                        for hh in range(2):
                            h = 2 * p + hh
                            nc.tensor.matmul(
                                avs[p][64 * hh:64 * hh + 64, w_:SBW],
                                vt[j_][:, 64 * h: 64 * h + 64],
                                pts_[p][:, hh * SBW + w_:(hh + 1) * SBW],
                                start=False, stop=lst,
                                tile_position=(0, 64 * hh),
                                skip_group_check=True,
                            )

                    av_pair(0)
                    av_pair(1)
                    for h in range(HPC):
                        nc.tensor.matmul(
                            sums[32 * h: 32 * h + 1, w_:SBW],
                            ones_sum[:, 0:1],
                            pts_[h // 2][:, (h % 2) * SBW + w_:
                                         (h % 2 + 1) * SBW],
                            start=False, stop=lst,
                            tile_position=(0, 32 * h),
                            skip_group_check=True,
                        )

                prev = None  # (pts, w, j) deferred by one iteration
                for j in range(njs):
                    w = 128 * (j - 4 * i) if j >= 4 * i else 0
                    cw = SBW - w
                    pts = []
                    for p in range(2):  # head pairs (0,1) and (2,3)
                        sc2 = accp.tile([128, 2 * SBW], F32, name="sc2",
                                        tag="acc")
                        for hh in range(2):
                            h = 2 * p + hh
                            dt_i, po = h // 2, 64 * (h % 2)
                            nc.tensor.matmul(
                                sc2[:, hh * SBW: hh * SBW + cw],
                                kt[dt_i][po:po + 64, j * TBW:(j + 1) * TBW],
                                qt[dt_i][po:po + 64,
                                         i * SBW + w: (i + 1) * SBW],
                                start=True,
                                stop=True,
                            )
                        pt_t = ptp.tile([128, 2 * SBW], BF16, name="ptile",
                                        tag="pt")
                        nc.scalar.activation(
                            pt_t.rearrange("q (g c) -> q g c",
                                           c=SBW)[:, :, w:SBW],
                            sc2.rearrange("q (g c) -> q g c",
                                          c=SBW)[:, :, 0:cw],
                            Exp,
                        )
                        if j >= 4 * i:  # diagonal: 0/1 triangular mask on PT
                            for hh in range(2):
                                zone = hh * SBW + w
                                nc.vector.tensor_mul(
                                    pt_t[:, zone: zone + 128],
                                    pt_t[:, zone: zone + 128],
                                    tri[:],
                                )
                        pts.append(pt_t)
                    if prev is not None:
                        av_mms(*prev)
                    prev = (pts, w, j)
                    if j >= 2 and drip:
                        emit_drip(drip.pop(0))
                for u in drip:
                    emit_drip(u)
                av_mms(*prev)

                # normalize: yt[p] s-block i = avs[p] / broadcast(sums).
                # The PE is kept busy through the DVE-heavy normalization:
                # V units for block i+1, or (last block) dummy warm matmuls
                # into the freed score slot so HAM stays at full clock for
                # the tail projection.
                rs_bf = rsp.tile([128, SBW], BF16, name="rs_bf", tag="rs")
                nc.vector.tensor_copy(rs_bf[0:97, :], sums[0:97, :])

                def pe_fill():
                    if vq:
                        emit_v_unit(vq.pop(0))
                    else:
                        dum = accp.tile([128, 2 * SBW], F32, name="dum",
                                        tag="acc")
                        nc.tensor.matmul(
                            dum[:, 0:512], wp[:, 0:128], wp[:, 0:512],
                            start=True, stop=True,
                        )

                pe_fill()
                for p in range(2):
                    # bc broadcast reuses the sums bank (free after rs_bf copy)
                    for hh in range(2):
                        h = 2 * p + hh
                        nc.tensor.matmul(
                            sums[64 * hh: 64 * hh + 64, :],
                            ones_bc[32 * h: 32 * h + 1, 0:64],
                            rs_bf[32 * h: 32 * h + 1, :],
                            start=True, stop=True,
                            tile_position=(32 * h, 64 * hh),
                            skip_group_check=True,
                        )
                    bcr = bcsp.tile([128, SBW], F32, name="bcr", tag="bcs")
                    nc.vector.reciprocal_approx_fast(bcr[:], sums[:])
                    nc.vector.tensor_mul(
                        yt[p][:, i * SBW:(i + 1) * SBW], avs[p][:], bcr[:],
                    )
                    pe_fill()
                for j16 in vq:
                    emit_v_unit(j16)
                units = [(i * SBW + st * 128, nb2)
                         for st in range(4) for nb2 in range(2)]
                if i < NSB - 1:
                    proj_pending = units
                else:
                    last_units = units

        # ---- phase C: last block's projection in its own psum pool ----
        with tc.tile_pool(name="tailp", bufs=4, space="PSUM") as tailp:
            emit_proj_units(last_units, tailp, "tl", split_ot=True)


def _build():
    global _BUILT
    if _BUILT is not None:
        return _BUILT
    nc = bacc.Bacc("TRN2", target_bir_lowering=False, debug=False,
                   num_devices=NCORES)
    d = {
        "xt": nc.dram_tensor("xt", [128, NEB * S], BF16, kind="ExternalInput").ap(),
        "wq": nc.dram_tensor("wq", [128, NEB * EL], BF16, kind="ExternalInput").ap(),
        "wk": nc.dram_tensor("wk", [128, NEB * EL], BF16, kind="ExternalInput").ap(),
        "wv": nc.dram_tensor("wv", [128, NEB * EL], BF16, kind="ExternalInput").ap(),
        "wp": nc.dram_tensor("wp", [128, 2 * E], BF16, kind="ExternalInput").ap(),
        "bq": nc.dram_tensor("bq", [128, 2], F32, kind="ExternalInput").ap(),
        "bk": nc.dram_tensor("bk", [128, 2], F32, kind="ExternalInput").ap(),
        "bv": nc.dram_tensor("bv", [128, EL], F32, kind="ExternalInput").ap(),
        "tri": nc.dram_tensor("tri", [128, 128], BF16, kind="ExternalInput").ap(),
        "out": nc.dram_tensor("out", [S, E], BF16, kind="ExternalOutput").ap(),
    }
    with tile.TileContext(nc) as tc:
        _emit(tc, nc, d)
    nc.compile()
    _BUILT = nc
    return _BUILT


def _blockify(a, pblk):
    """[N*pblk, M] -> [pblk, N*M] with block-column layout."""
    n = a.shape[0] // pblk
    return np.ascontiguousarray(
        a.reshape(n, pblk, a.shape[1]).transpose(1, 0, 2).reshape(pblk, -1)
    )


def _prep_core(c, x, Wq, bq, Wk, bk, Wv, bv, Wp):
    b, g = c // 4, c % 4
    lo = EL * g
    bf = ml_dtypes.bfloat16

    xT = np.ascontiguousarray(x[b].T)                        # [E, S]
    wqT = np.ascontiguousarray(Wq[lo:lo + EL, :].T) * SCALE  # [E, 256]
    wkT = np.ascontiguousarray(Wk[lo:lo + EL, :].T)
    wvT = np.ascontiguousarray(Wv[lo:lo + EL, :].T)
    wpT = np.ascontiguousarray(Wp[:, lo:lo + EL].T)          # [256, E]

    col = np.arange(128, dtype=np.int64)
    tri = np.where(col[None, :] >= np.arange(128)[:, None], 1.0, 0.0)

    return {
        "xt": _blockify(xT, 128).astype(bf),
        "wq": _blockify(wqT, 128).astype(bf),
        "wk": _blockify(wkT, 128).astype(bf),
        "wv": _blockify(wvT, 128).astype(bf),
        "wp": _blockify(wpT, 128).astype(bf),
        "bq": np.ascontiguousarray(
            (bq[lo:lo + EL] * SCALE).reshape(2, 128).T).astype(np.float32),
        "bk": np.ascontiguousarray(
            bk[lo:lo + EL].reshape(2, 128).T).astype(np.float32),
        "bv": np.ascontiguousarray(
            np.broadcast_to(bv[lo:lo + EL], (128, EL))).astype(np.float32),
        "tri": tri.astype(bf),
    }


def run(inputs, trace=False):
    """Run on hardware. Returns (out [B,S,E] f32, exec_time_ns, results)."""
    x = np.asarray(inputs["x"], np.float32)
    Wq = np.asarray(inputs["Wq"], np.float32)
    bq = np.asarray(inputs["bq"], np.float32)
    Wk = np.asarray(inputs["Wk"], np.float32)
    bk = np.asarray(inputs["bk"], np.float32)
    Wv = np.asarray(inputs["Wv"], np.float32)
    bv = np.asarray(inputs["bv"], np.float32)
    Wp = np.asarray(inputs["Wp"], np.float32)
    bp = np.asarray(inputs["bp"], np.float32)

    nc = _build()
    in_maps = [
        _prep_core(c, x, Wq, bq, Wk, bk, Wv, bv, Wp) for c in range(NCORES)
    ]
    kwargs = {}
    if trace:
        try:
            import ntff_shim
            ntff_shim.install()
        except Exception:
            pass
        kwargs["trace"] = True
    res = bass_utils.run_bass_kernel_spmd(
        nc, in_maps, list(range(NCORES)), **kwargs
    )
    out = np.empty((B, S, E), np.float32)
    for b in range(B):
        acc = res.results[4 * b]["out"].astype(np.float32)
        for g in range(1, 4):
            acc = acc + res.results[4 * b + g]["out"].astype(np.float32)
        out[b] = acc + bp[None, :]
    return out, res.exec_time_ns, res


def kernel(**inputs):
    out, _, _ = run(inputs, trace=False)
    return out
